# revision 10
# baseline (speedup 1.0000x reference)
"""Trainium2 Bass kernel for ContinuousAttention (self-keyed RoPE attention,
strictly-causal masked scores, no softmax).

Reference computation (B=2, NH=16, T=2048, N=256, fp32):
    QR = rope(Q)                      # interleaved-pair RoPE, freqs quantized in pairs
    S  = QR @ QR^T                    # per (b, h); K input is unused by the module
    O  = (S * strict_causal_mask) @ V

Sharding: 32 (b*nh) heads over 8 NeuronCores, 4 heads per core; no
communication.  Each core runs an identical program on its head slice.

v2 design (fp16 matmul operands, fp32 PSUM accumulation):
  - Host ships Q, pair-swapped Q, and V in fp16, plus transposed RoPE tables.
  - Per head, xbar DMA-transposes Q / Qswap chunks straight from DRAM into
    (n, t) layout; RoPE is then 3 dense DVE ops per 128-partition chunk:
        QRT = QT * cosT + QswapT * sinT_signed
  - matmul1: T_ij = S_ij^T strips (stationary = QRT j-block, moving = QRT
    256-wide t-group), only causal-triangle groups; PSUM->SBUF copies cast to
    fp16 and apply the strict mask on diagonal blocks.
  - matmul2: O_i = sum_{j<=i} T_ij^T @ V_j accumulated in PSUM (fp32).
  - O tiles -> fp32 staging tile -> one DMA per head.
"""

import math
import sys

import numpy as np

if "/opt/trn_rl_repo" not in sys.path:
    sys.path.insert(0, "/opt/trn_rl_repo")

import concourse.bass as bass
import concourse.mybir as mybir
import concourse.tile as tile
from concourse.bass_utils import run_bass_kernel_spmd

B, NH, T, N = 2, 16, 2048, 256
THETA = 2 ** 16
N_CORES = 8
H_PER_CORE = (B * NH) // N_CORES

F32 = mybir.dt.float32
FP16 = mybir.dt.float16
MULT = mybir.AluOpType.mult
ADD = mybir.AluOpType.add
HF = np.float16


def _split_overloaded_waits(nc, max_waits=1):
    """walrus in this container rejects >1 sync-wait per instruction; move
    extra waits onto preceding same-engine NoOps (semantically identical)."""
    n_split = 0
    for f in nc.m.functions:
        for bb in f.blocks:
            new_list = []
            changed = False
            for ins in bb.instructions:
                si = getattr(ins, "sync_info", None)
                if si is not None and len(si.on_wait) > max_waits:
                    waits = list(si.on_wait)
                    extra, keep = waits[:-max_waits], waits[-max_waits:]
                    k = 0
                    while extra:
                        chunk, extra = extra[:max_waits], extra[max_waits:]
                        nop = mybir.InstNoOp(
                            name=f"{ins.name}_wsplit{k}", ins=[], outs=[]
                        )
                        nop.engine = ins.engine
                        nop.sync_info = mybir.SyncInfo(on_wait=chunk, on_update=[])
                        new_list.append(nop)
                        k += 1
                    ins.sync_info = mybir.SyncInfo(
                        on_wait=keep, on_update=list(si.on_update)
                    )
                    changed = True
                    n_split += 1
                new_list.append(ins)
            if changed:
                bb.instructions = new_list
    return n_split


def rope_tables(t=T, n=N, dtype=np.float32):
    """cos table and sign-folded sin table, natural (t, n) layout."""
    idx = np.floor(np.arange(n, dtype=dtype) / dtype(2.0)) * dtype(2.0)
    freqs = (
        dtype(1.0) / (dtype(THETA) ** (idx / dtype(n))) / dtype(2.0 * math.pi)
    ).astype(dtype)
    phases = np.arange(t, dtype=dtype)[:, None] * freqs[None, :]
    ph = (phases % dtype(1.0)) * dtype(2.0 * math.pi)
    cos = np.cos(ph).astype(dtype)
    sin = np.sin(ph).astype(dtype)
    sin_a = sin.copy()
    sin_a[:, 0::2] *= dtype(-1.0)  # fold the rotate-pair sign into sin
    return cos, sin_a


def build_nc(h_per_core=H_PER_CORE, t=T, n=N, waitsplit=True):
    """v4: RoPE precomputed on host (ships QR directly) — the device does
    only the two chained matmuls.  mm1 runs trimmed-causal in <=512-wide
    moving chunks (80 matmuls/head, zero wasted blocks); mm2 for a 512-row
    t-chunk is interleaved right after that chunk's mm1 so PSUM-drain
    pressure on Vector/Scalar stays smooth and the PE never starves."""
    assert n == 256 and t % 512 == 0
    nt = t // 128   # 128-row s-blocks (16)
    ntc = t // 512  # 512-wide t-chunks (4)
    nc = bass.Bass("TRN2", target_bir_lowering=False, debug=False)

    qrtd = nc.dram_tensor("qrt", [h_per_core, n, t], FP16, kind="ExternalInput").ap()
    v = nc.dram_tensor("v", [h_per_core, t, n], FP16, kind="ExternalInput").ap()
    o = nc.dram_tensor("o", [h_per_core, t, n], F32, kind="ExternalOutput").ap()

    with tile.TileContext(nc) as tc:
        with (
            tc.tile_pool(name="const", bufs=1) as cpool,
            tc.tile_pool(name="qrt", bufs=2) as qrtpool,
            tc.tile_pool(name="strips", bufs=2) as strippool,
            tc.tile_pool(name="vh", bufs=2) as vpool,
            tc.tile_pool(name="oh", bufs=2) as ohpool,
            tc.tile_pool(name="sps", bufs=5, space="PSUM") as spool,
            tc.tile_pool(name="ops", bufs=3, space="PSUM") as opool,
        ):
            # diag-chunk mask, (s, t) orientation: cols<128 keep iff t>s,
            # cols>=128 always keep (t provably > s there)
            mask = cpool.tile([128, 512], F32)
            nc.gpsimd.memset(mask, 1.0)
            nc.gpsimd.affine_select(
                out=mask[:, 0:128],
                in_=mask[:, 0:128],
                compare_op=mybir.AluOpType.is_ge,
                fill=0.0,
                base=-1,
                pattern=[[1, 128]],
                channel_multiplier=-1,
            )

            dr = 0  # full-chunk drain round robin (scalar / vector)
            do = 0  # O drain round robin
            for h in range(h_per_core):
                # ---- DMA rotated Q, (n, t) layout, 512-col segments ----
                qrt = [
                    qrtpool.tile([128, t], FP16, tag=f"qrt{c}", name=f"qrt{c}")
                    for c in range(2)
                ]
                for s in range(ntc):
                    tsl = slice(s * 512, (s + 1) * 512)
                    for c in range(2):
                        psl = slice(c * 128, (c + 1) * 128)
                        # two issue queues so the c=0/c=1 descriptor writes
                        # (~0.7us each) don't serialize ahead of the first
                        # matmul
                        eng = nc.sync if c == 0 else nc.scalar
                        eng.dma_start(out=qrt[c][:, tsl], in_=qrtd[h][psl, tsl])
                vh = vpool.tile([128, nt * n], FP16, tag="vh", name="vh")
                nc.sync.dma_start(
                    out=vh.rearrange("p (t n) -> p t n", n=n),
                    in_=v[h].rearrange("(t p) n -> p t n", p=128),
                )

                strips = [
                    strippool.tile(
                        [128, t - 128 * j], FP16,
                        tag=f"strip{j}", name=f"strip{j}",
                    )
                    for j in range(nt)
                ]
                oh = ohpool.tile([128, nt * n], F32, tag="oh", name="oh")

                def mm2(po, i, j):
                    loff = 128 * (i - j)
                    nc.tensor.matmul(
                        po,
                        lhsT=strips[j][:, loff:loff + 128],
                        rhs=vh[:, j * n:(j + 1) * n],
                        start=(j == 0),
                        stop=(j == i),
                    )

                def o_writeback(i, po):
                    nonlocal do
                    dst = oh[:, i * n:(i + 1) * n]
                    if do % 2 == 0:
                        nc.vector.tensor_copy(out=dst, in_=po)
                    else:
                        nc.scalar.copy(out=dst, in_=po)
                    do += 1

                for tcx in range(ntc):
                    # ---- mm1 for t-chunk tcx: strips[j][:, tc cols] ----
                    base_t = 512 * tcx
                    for j in range(4 * tcx + 4):
                        diag = j >= 4 * tcx
                        off = 128 * (j - 4 * tcx) if diag else 0
                        width = 512 - off
                        col0 = base_t + off
                        ps = spool.tile([128, 512], F32, name="ps")
                        for c in range(2):
                            nc.tensor.matmul(
                                ps[:, :width],
                                lhsT=qrt[c][:, j * 128:(j + 1) * 128],
                                rhs=qrt[c][:, col0:base_t + 512],
                                start=(c == 0),
                                stop=(c == 1),
                            )
                        dst = strips[j][:, col0 - 128 * j:col0 - 128 * j + width]
                        if diag:  # fused strict-causal mask + cast drain
                            nc.vector.tensor_tensor(
                                out=dst, in0=ps[:, :width],
                                in1=mask[:, :width], op=MULT,
                            )
                        else:
                            if dr % 2 == 0:
                                nc.scalar.copy(out=dst, in_=ps[:, :width])
                            else:
                                nc.vector.tensor_copy(out=dst, in_=ps[:, :width])
                            dr += 1

                    # ---- mm2 for the four i-blocks of this t-chunk ----
                    for i0 in range(4 * tcx, 4 * tcx + 4, 2):
                        i1 = i0 + 1
                        po0 = opool.tile([128, n], F32, name="po")
                        po1 = opool.tile([128, n], F32, name="po")
                        for j in range(i0 + 1):
                            mm2(po0, i0, j)
                            mm2(po1, i1, j)
                        mm2(po1, i1, i1)
                        o_writeback(i0, po0)
                        o_writeback(i1, po1)
                        # drain output per i-pair: keeps the final DMA small
                        # so the kernel tail is short
                        isl = slice(i0 * 128, (i1 + 1) * 128)
                        csl = slice(i0 * n, (i1 + 1) * n)
                        nc.scalar.dma_start(
                            out=o[h][isl].rearrange("(t p) n -> p t n", p=128),
                            in_=oh[:, csl].rearrange("p (t n) -> p t n", n=n),
                        )

    if waitsplit:
        _split_overloaded_waits(nc)
    return nc


_NC_CACHE = {}


def get_nc(h_per_core=H_PER_CORE, t=T, n=N):
    key = (h_per_core, t, n)
    if key not in _NC_CACHE:
        _NC_CACHE[key] = build_nc(h_per_core, t, n)
    return _NC_CACHE[key]


def make_in_maps(Q, V, n_cores=N_CORES):
    b, nh, t, n = Q.shape
    h_per_core = (b * nh) // n_cores
    qf = np.asarray(Q, dtype=np.float32).reshape(b * nh, t, n)
    vf = np.asarray(V, dtype=np.float32).reshape(b * nh, t, n)
    # RoPE on host in fp32 (input prep, like the layout transposes):
    # qr = q * cos + pairswap(q) * sign-folded-sin
    qsw = qf.reshape(b * nh, t, n // 2, 2)[..., ::-1].reshape(b * nh, t, n)
    cos, sin_a = rope_tables(t, n)
    qr = (qf * cos + qsw * sin_a).astype(HF)
    # pre-transposed (n, t) layout so the device needs only plain DMAs
    qrtb = np.ascontiguousarray(qr.transpose(0, 2, 1))
    vb = vf.astype(HF)
    in_maps = []
    for c in range(n_cores):
        sl = slice(c * h_per_core, (c + 1) * h_per_core)
        in_maps.append(
            {
                "qrt": np.ascontiguousarray(qrtb[sl]),
                "v": np.ascontiguousarray(vb[sl]),
            }
        )
    return in_maps


def kernel(Q, K, V):
    """Full-input entry point: Q, K, V are (B, NH, T, N) float32 numpy arrays.
    K is unused (the module self-keys attention on rotated Q)."""
    Q = np.asarray(Q)
    V = np.asarray(V)
    b, nh, t, n = Q.shape
    nc = get_nc((b * nh) // N_CORES, t, n)
    in_maps = make_in_maps(Q, V, N_CORES)
    res = None
    last_err = None
    for attempt in range(3):  # retry transient device/runtime failures
        try:
            res = run_bass_kernel_spmd(
                nc, in_maps, core_ids=list(range(N_CORES)), trace=False
            )
            break
        except Exception as e:  # e.g. NRT_EXEC_UNIT_UNRECOVERABLE after a
            last_err = e  # wedged prior run; a clean retry usually recovers
            import time as _time

            _time.sleep(2.0 * (attempt + 1))
    if res is None:
        raise last_err
    outs = [res.results[c]["o"] for c in range(N_CORES)]
    out = np.concatenate(outs, axis=0).reshape(b, nh, t, n)
    return out.astype(np.float32)



# revision 13
# speedup vs baseline: 1.0040x; 1.0040x over previous
"""Trainium2 Bass kernel for ContinuousAttention (self-keyed RoPE attention,
strictly-causal masked scores, no softmax).

Reference computation (B=2, NH=16, T=2048, N=256, fp32):
    QR = rope(Q)                      # interleaved-pair RoPE, freqs quantized in pairs
    S  = QR @ QR^T                    # per (b, h); K input is unused by the module
    O  = (S * strict_causal_mask) @ V

Sharding: 32 (b*nh) heads over 8 NeuronCores, 4 heads per core; no
communication.  Each core runs an identical program on its head slice.

v2 design (fp16 matmul operands, fp32 PSUM accumulation):
  - Host ships Q, pair-swapped Q, and V in fp16, plus transposed RoPE tables.
  - Per head, xbar DMA-transposes Q / Qswap chunks straight from DRAM into
    (n, t) layout; RoPE is then 3 dense DVE ops per 128-partition chunk:
        QRT = QT * cosT + QswapT * sinT_signed
  - matmul1: T_ij = S_ij^T strips (stationary = QRT j-block, moving = QRT
    256-wide t-group), only causal-triangle groups; PSUM->SBUF copies cast to
    fp16 and apply the strict mask on diagonal blocks.
  - matmul2: O_i = sum_{j<=i} T_ij^T @ V_j accumulated in PSUM (fp32).
  - O tiles -> fp32 staging tile -> one DMA per head.
"""

import math
import sys

import numpy as np

if "/opt/trn_rl_repo" not in sys.path:
    sys.path.insert(0, "/opt/trn_rl_repo")

import concourse.bass as bass
import concourse.mybir as mybir
import concourse.tile as tile
from concourse.bass_utils import run_bass_kernel_spmd

B, NH, T, N = 2, 16, 2048, 256
THETA = 2 ** 16
N_CORES = 8
H_PER_CORE = (B * NH) // N_CORES

F32 = mybir.dt.float32
FP16 = mybir.dt.float16
MULT = mybir.AluOpType.mult
ADD = mybir.AluOpType.add
HF = np.float16


def _split_overloaded_waits(nc, max_waits=1):
    """walrus in this container rejects >1 sync-wait per instruction; move
    extra waits onto preceding same-engine NoOps (semantically identical)."""
    n_split = 0
    for f in nc.m.functions:
        for bb in f.blocks:
            new_list = []
            changed = False
            for ins in bb.instructions:
                si = getattr(ins, "sync_info", None)
                if si is not None and len(si.on_wait) > max_waits:
                    waits = list(si.on_wait)
                    extra, keep = waits[:-max_waits], waits[-max_waits:]
                    k = 0
                    while extra:
                        chunk, extra = extra[:max_waits], extra[max_waits:]
                        nop = mybir.InstNoOp(
                            name=f"{ins.name}_wsplit{k}", ins=[], outs=[]
                        )
                        nop.engine = ins.engine
                        nop.sync_info = mybir.SyncInfo(on_wait=chunk, on_update=[])
                        new_list.append(nop)
                        k += 1
                    ins.sync_info = mybir.SyncInfo(
                        on_wait=keep, on_update=list(si.on_update)
                    )
                    changed = True
                    n_split += 1
                new_list.append(ins)
            if changed:
                bb.instructions = new_list
    return n_split


def rope_tables(t=T, n=N, dtype=np.float32):
    """cos table and sign-folded sin table, natural (t, n) layout."""
    idx = np.floor(np.arange(n, dtype=dtype) / dtype(2.0)) * dtype(2.0)
    freqs = (
        dtype(1.0) / (dtype(THETA) ** (idx / dtype(n))) / dtype(2.0 * math.pi)
    ).astype(dtype)
    phases = np.arange(t, dtype=dtype)[:, None] * freqs[None, :]
    ph = (phases % dtype(1.0)) * dtype(2.0 * math.pi)
    cos = np.cos(ph).astype(dtype)
    sin = np.sin(ph).astype(dtype)
    sin_a = sin.copy()
    sin_a[:, 0::2] *= dtype(-1.0)  # fold the rotate-pair sign into sin
    return cos, sin_a


def build_nc(h_per_core=H_PER_CORE, t=T, n=N, waitsplit=True):
    """v4: RoPE precomputed on host (ships QR directly) — the device does
    only the two chained matmuls.  mm1 runs trimmed-causal in <=512-wide
    moving chunks (80 matmuls/head, zero wasted blocks); mm2 for a 512-row
    t-chunk is interleaved right after that chunk's mm1 so PSUM-drain
    pressure on Vector/Scalar stays smooth and the PE never starves."""
    assert n == 256 and t % 512 == 0
    nt = t // 128   # 128-row s-blocks (16)
    ntc = t // 512  # 512-wide t-chunks (4)
    nc = bass.Bass("TRN2", target_bir_lowering=False, debug=False)

    qrtd = nc.dram_tensor("qrt", [h_per_core, n, t], FP16, kind="ExternalInput").ap()
    v = nc.dram_tensor("v", [h_per_core, t, n], FP16, kind="ExternalInput").ap()
    o = nc.dram_tensor("o", [h_per_core, t, n], F32, kind="ExternalOutput").ap()

    with tile.TileContext(nc) as tc:
        with (
            tc.tile_pool(name="const", bufs=1) as cpool,
            tc.tile_pool(name="qrt", bufs=2) as qrtpool,
            tc.tile_pool(name="strips", bufs=2) as strippool,
            tc.tile_pool(name="vh", bufs=2) as vpool,
            tc.tile_pool(name="oh", bufs=2) as ohpool,
            tc.tile_pool(name="sps", bufs=5, space="PSUM") as spool,
            tc.tile_pool(name="ops", bufs=3, space="PSUM") as opool,
        ):
            # diag-chunk mask, (s, t) orientation: cols<128 keep iff t>s,
            # cols>=128 always keep (t provably > s there)
            mask = cpool.tile([128, 512], F32)
            nc.gpsimd.memset(mask, 1.0)
            nc.gpsimd.affine_select(
                out=mask[:, 0:128],
                in_=mask[:, 0:128],
                compare_op=mybir.AluOpType.is_ge,
                fill=0.0,
                base=-1,
                pattern=[[1, 128]],
                channel_multiplier=-1,
            )

            # HAM warmup: ~3.4us of dummy PE activity while head 0's input
            # DMAs are still in flight, so real matmuls start un-throttled
            # (2.4 GHz) instead of paying the cold 1.2 GHz window.
            warm = spool.tile([128, 512], F32, tag="ps", name="warm")
            for _ in range(2):  # fp32 = 4 cyc/row -> ~1.7us each cold
                nc.tensor.matmul(
                    warm, lhsT=mask[:, 0:128], rhs=mask, start=True, stop=True
                )

            dr = 0  # full-chunk drain round robin (scalar / vector)
            do = 0  # O drain round robin
            for h in range(h_per_core):
                # ---- DMA rotated Q, (n, t) layout, 512-col segments ----
                qrt = [
                    qrtpool.tile([128, t], FP16, tag=f"qrt{c}", name=f"qrt{c}")
                    for c in range(2)
                ]
                # all input DMAs on the sync queue (scalar stays pure-drain);
                # vh last — it is first needed ~8 matmuls into mm1
                for s in range(ntc):
                    tsl = slice(s * 512, (s + 1) * 512)
                    for c in range(2):
                        psl = slice(c * 128, (c + 1) * 128)
                        nc.sync.dma_start(
                            out=qrt[c][:, tsl], in_=qrtd[h][psl, tsl]
                        )
                vh = vpool.tile([128, nt * n], FP16, tag="vh", name="vh")
                nc.sync.dma_start(
                    out=vh.rearrange("p (t n) -> p t n", n=n),
                    in_=v[h].rearrange("(t p) n -> p t n", p=128),
                )

                strips = [
                    strippool.tile(
                        [128, t - 128 * j], FP16,
                        tag=f"strip{j}", name=f"strip{j}",
                    )
                    for j in range(nt)
                ]
                oh = ohpool.tile([128, nt * n], F32, tag="oh", name="oh")

                def mm2(po, i, j):
                    loff = 128 * (i - j)
                    nc.tensor.matmul(
                        po,
                        lhsT=strips[j][:, loff:loff + 128],
                        rhs=vh[:, j * n:(j + 1) * n],
                        start=(j == 0),
                        stop=(j == i),
                    )

                def o_writeback(i, po):
                    nonlocal do
                    dst = oh[:, i * n:(i + 1) * n]
                    if do % 2 == 0:
                        nc.vector.tensor_copy(out=dst, in_=po)
                    else:
                        nc.scalar.copy(out=dst, in_=po)
                    do += 1

                for tcx in range(ntc):
                    # ---- mm1 for t-chunk tcx: strips[j][:, tc cols] ----
                    base_t = 512 * tcx
                    for j in range(4 * tcx + 4):
                        diag = j >= 4 * tcx
                        off = 128 * (j - 4 * tcx) if diag else 0
                        width = 512 - off
                        col0 = base_t + off
                        ps = spool.tile([128, 512], F32, name="ps")
                        for c in range(2):
                            nc.tensor.matmul(
                                ps[:, :width],
                                lhsT=qrt[c][:, j * 128:(j + 1) * 128],
                                rhs=qrt[c][:, col0:base_t + 512],
                                start=(c == 0),
                                stop=(c == 1),
                            )
                        dst = strips[j][:, col0 - 128 * j:col0 - 128 * j + width]
                        if diag:  # fused strict-causal mask + cast drain
                            nc.vector.tensor_tensor(
                                out=dst, in0=ps[:, :width],
                                in1=mask[:, :width], op=MULT,
                            )
                        else:
                            if dr % 2 == 0:
                                nc.scalar.copy(out=dst, in_=ps[:, :width])
                            else:
                                nc.vector.tensor_copy(out=dst, in_=ps[:, :width])
                            dr += 1

                    # ---- mm2 for the four i-blocks of this t-chunk ----
                    for i0 in range(4 * tcx, 4 * tcx + 4, 2):
                        i1 = i0 + 1
                        po0 = opool.tile([128, n], F32, name="po")
                        po1 = opool.tile([128, n], F32, name="po")
                        for j in range(i0 + 1):
                            mm2(po0, i0, j)
                            mm2(po1, i1, j)
                        mm2(po1, i1, i1)
                        o_writeback(i0, po0)
                        o_writeback(i1, po1)
                        # drain output per i-pair: keeps the final DMA small
                        # so the kernel tail is short
                        isl = slice(i0 * 128, (i1 + 1) * 128)
                        csl = slice(i0 * n, (i1 + 1) * n)
                        nc.scalar.dma_start(
                            out=o[h][isl].rearrange("(t p) n -> p t n", p=128),
                            in_=oh[:, csl].rearrange("p (t n) -> p t n", n=n),
                        )

    if waitsplit:
        _split_overloaded_waits(nc)
    return nc


_NC_CACHE = {}


def get_nc(h_per_core=H_PER_CORE, t=T, n=N):
    key = (h_per_core, t, n)
    if key not in _NC_CACHE:
        _NC_CACHE[key] = build_nc(h_per_core, t, n)
    return _NC_CACHE[key]


def make_in_maps(Q, V, n_cores=N_CORES):
    b, nh, t, n = Q.shape
    h_per_core = (b * nh) // n_cores
    qf = np.asarray(Q, dtype=np.float32).reshape(b * nh, t, n)
    vf = np.asarray(V, dtype=np.float32).reshape(b * nh, t, n)
    # RoPE on host in fp32 (input prep, like the layout transposes):
    # qr = q * cos + pairswap(q) * sign-folded-sin
    qsw = qf.reshape(b * nh, t, n // 2, 2)[..., ::-1].reshape(b * nh, t, n)
    cos, sin_a = rope_tables(t, n)
    qr = (qf * cos + qsw * sin_a).astype(HF)
    # pre-transposed (n, t) layout so the device needs only plain DMAs
    qrtb = np.ascontiguousarray(qr.transpose(0, 2, 1))
    vb = vf.astype(HF)
    in_maps = []
    for c in range(n_cores):
        sl = slice(c * h_per_core, (c + 1) * h_per_core)
        in_maps.append(
            {
                "qrt": np.ascontiguousarray(qrtb[sl]),
                "v": np.ascontiguousarray(vb[sl]),
            }
        )
    return in_maps


def kernel(Q, K, V):
    """Full-input entry point: Q, K, V are (B, NH, T, N) float32 numpy arrays.
    K is unused (the module self-keys attention on rotated Q)."""
    Q = np.asarray(Q)
    V = np.asarray(V)
    b, nh, t, n = Q.shape
    nc = get_nc((b * nh) // N_CORES, t, n)
    in_maps = make_in_maps(Q, V, N_CORES)
    res = None
    last_err = None
    for attempt in range(3):  # retry transient device/runtime failures
        try:
            res = run_bass_kernel_spmd(
                nc, in_maps, core_ids=list(range(N_CORES)), trace=False
            )
            break
        except Exception as e:  # e.g. NRT_EXEC_UNIT_UNRECOVERABLE after a
            last_err = e  # wedged prior run; a clean retry usually recovers
            import time as _time

            _time.sleep(2.0 * (attempt + 1))
    if res is None:
        raise last_err
    outs = [res.results[c]["o"] for c in range(N_CORES)]
    out = np.concatenate(outs, axis=0).reshape(b, nh, t, n)
    return out.astype(np.float32)

